# revision 22
# baseline (speedup 1.0000x reference)
"""Trainium2 Bass kernel for nn_CADense (context-adaptive low-rank dense layer).

Computes, for the full batch:
    s_mod = s + context @ w          # [B, R]
    low   = (data @ u) * s_mod       # [B, R]
    out   = relu(low @ v.T + 2*bias) # [B, UNITS]

Sharding: data-parallel over batch across 8 NeuronCores; u/s/v/w/bias
replicated. Each core runs the same Bass program on its 1024-row shard.

All heavy streams are bf16: inputs are downcast host-side into pre-tiled,
fully-contiguous per-DMA slabs (partition-major, 4KB contiguous per
partition line), and the output is stored bf16 and upcast host-side.
This halves HBM traffic vs f32 (11.8 MB/core) and runs the PE at
1 cycle/row with hardware fast-weight-load, so DMA (~33 us) and PE
(~31 us) land together at the roofline ridge.

Compute per 512-row batch tile, in the transposed domain:
    pd[r, b]   = (u.T @ data.T)[r, b]          (16 k-chunk accumulation)
    smod[r, b] = s[r] + (w.T @ ctx.T)[r, b]    (4 c-chunk accumulation)
    lowT       = pd * smod                      (DVE, bf16 out)
    out[b, :]  = relu(lowT.T @ v.T)             (per 128-row chunk)
ReLU evacuation of the output PSUM alternates scalar/vector engines and
stores ride the gpsimd SWDGE ring; loads are split across the sync and
scalar HWDGE rings. bias is all-zero in this model configuration; a
separate program variant folds nonzero bias in as K=1 rank-1 matmuls.
"""

import os
import sys
from contextlib import ExitStack

import numpy as np

try:
    import ml_dtypes
except ImportError:  # pragma: no cover
    ml_dtypes = None


def _ensure_concourse():
    try:
        import concourse  # noqa: F401
    except ImportError:
        for p in ("/opt/trn_rl_repo", "/root/.axon_site/_ro/trn_rl_repo"):
            if os.path.isdir(p) and p not in sys.path:
                sys.path.insert(0, p)


_ensure_concourse()

import concourse.tile as tile  # noqa: E402
from concourse import bacc, mybir  # noqa: E402
from concourse.bass_utils import run_bass_kernel_spmd  # noqa: E402

if ml_dtypes is None:
    import ml_dtypes  # noqa: E402  (bundled with concourse deps)

NCORES = 8
B, N_IN, UNITS, RANK, CCTX = 8192, 2048, 2048, 256, 512
NB = B // NCORES  # batch rows per core
P = 128
BT = 512  # batch tile (free dim of T-domain matmuls)
NBT = NB // BT  # batch tiles per core
KC = N_IN // P  # 16 contraction chunks for data @ u
CC = CCTX // P  # 4 contraction chunks for context @ w
RC = RANK // P  # 2 rank chunks
MS = 512  # output units slice width
NMS = UNITS // MS  # 4 unit slices
NQ = KC // 4  # dataT slab count per batch tile (4 k-chunks each)
N_WARMUP_MM = 18

F32 = mybir.dt.float32
BF16 = mybir.dt.bfloat16
BF16_NP = ml_dtypes.bfloat16


def _emit(nc, tc, ctx, with_bias):
    # Host-pretiled bf16 slabs; every DMA source is fully contiguous.
    # (0,0) is split into two half-slabs so the PE can start sooner.
    d_dataT = {
        (q, t): nc.dram_tensor(f"dataT{q}_{t}", [P, 4 * BT], BF16, kind="ExternalInput")
        for q in range(NQ)
        for t in range(NBT)
        if (q, t) != (0, 0)
    }
    d_dT00a = nc.dram_tensor("dataT0_0a", [P, 2 * BT], BF16, kind="ExternalInput")
    d_dT00b = nc.dram_tensor("dataT0_0b", [P, 2 * BT], BF16, kind="ExternalInput")
    d_ctxT = {
        t: nc.dram_tensor(f"ctxT{t}", [P, CC * BT], BF16, kind="ExternalInput")
        for t in range(NBT)
    }
    d_u = {
        uq: nc.dram_tensor(f"u{uq}", [P, 4 * RANK], BF16, kind="ExternalInput")
        for uq in range(4)
    }
    d_s = nc.dram_tensor("s", [P, RC], F32, kind="ExternalInput")
    d_vT = nc.dram_tensor("vT", [P, RC * UNITS], BF16, kind="ExternalInput")
    d_w = nc.dram_tensor("w", [P, CC * RANK], BF16, kind="ExternalInput")
    d_out = nc.dram_tensor("out", [NB, UNITS], BF16, kind="ExternalOutput")
    if with_bias:
        d_bias = nc.dram_tensor("bias2", [1, UNITS], BF16, kind="ExternalInput")

    singles = ctx.enter_context(tc.tile_pool(name="singles", bufs=1))
    du_psum = ctx.enter_context(tc.tile_pool(name="du_psum", bufs=2, space="PSUM"))
    # smod and output-stage PSUM tiles share one 6-bank cycle so mm2 is
    # never gated waiting for a just-evacuated bank.
    o_psum = ctx.enter_context(tc.tile_pool(name="o_psum", bufs=6, space="PSUM"))
    dTpool = ctx.enter_context(tc.tile_pool(name="dataT", bufs=1))
    cTpool = ctx.enter_context(tc.tile_pool(name="ctxT", bufs=2))
    lowpool = ctx.enter_context(tc.tile_pool(name="lowT", bufs=2))
    smodpool = ctx.enter_context(tc.tile_pool(name="smod", bufs=4))
    opool = ctx.enter_context(tc.tile_pool(name="outsb", bufs=3))

    # HAM warm-up fodder while the first loads stream.
    wu_a = singles.tile([P, P], BF16)
    nc.vector.memset(wu_a[:], 1.0)
    wu_b = singles.tile([P, 256], BF16)
    nc.vector.memset(wu_b[:], 1.0)

    # ---- input DMA queues ----------------------------------------------
    # Both HWDGE rings front-load the bytes that gate the first mm2
    # (u, dataT bt0, w, ctx0); everything else queues behind them.
    u_t = [singles.tile([P, 4 * RANK], BF16, name=f"uq{uq}") for uq in range(4)]
    dT_t = {
        (q, t): dTpool.tile([P, 4 * BT], BF16, tag=f"dT{q}_{t}", name=f"dT{q}_{t}")
        for q in range(NQ)
        for t in range(NBT)
    }
    w_sb = singles.tile([P, CC * RANK], BF16)
    ctxT_t = {t: cTpool.tile([P, CC * BT], BF16, tag="ctxT", name=f"ctxT{t}") for t in range(NBT)}
    s_sb = singles.tile([P, RC], F32)
    vT_sb = singles.tile([P, RC * UNITS], BF16)

    # Both rings deliver in exact PE-consumption order: all bt0 + vT
    # bytes strictly ahead of any bt1 bytes, vT split ms-granular so
    # mm2(0, ms) is never gated on the whole 1 MiB slab.
    # sync ring: q0/q1 of bt0, vT ms0/ms1, ctx1, then q0/q2 of bt1.
    nc.sync.dma_start(out=u_t[0][:], in_=d_u[0].ap())
    nc.sync.dma_start(out=dT_t[(0, 0)][:, 0 : 2 * BT], in_=d_dT00a.ap())
    nc.sync.dma_start(out=dT_t[(0, 0)][:, 2 * BT : 4 * BT], in_=d_dT00b.ap())
    nc.sync.dma_start(out=u_t[1][:], in_=d_u[1].ap())
    nc.sync.dma_start(out=dT_t[(1, 0)][:], in_=d_dataT[(1, 0)].ap())
    nc.sync.dma_start(out=vT_sb[:, 0:1024], in_=d_vT.ap()[:, 0:1024])
    nc.sync.dma_start(out=vT_sb[:, 1024:2048], in_=d_vT.ap()[:, 1024:2048])
    nc.sync.dma_start(out=ctxT_t[1][:], in_=d_ctxT[1].ap())
    nc.sync.dma_start(out=dT_t[(0, 1)][:], in_=d_dataT[(0, 1)].ap())
    nc.sync.dma_start(out=dT_t[(2, 1)][:], in_=d_dataT[(2, 1)].ap())

    # scalar ring: smod-0 inputs, q2/q3 of bt0, vT ms2/ms3, q1/q3 of bt1.
    nc.scalar.dma_start(out=w_sb[:], in_=d_w.ap())
    nc.scalar.dma_start(out=ctxT_t[0][:], in_=d_ctxT[0].ap())
    nc.scalar.dma_start(out=u_t[2][:], in_=d_u[2].ap())
    nc.scalar.dma_start(out=dT_t[(2, 0)][:], in_=d_dataT[(2, 0)].ap())
    nc.scalar.dma_start(out=u_t[3][:], in_=d_u[3].ap())
    nc.scalar.dma_start(out=dT_t[(3, 0)][:], in_=d_dataT[(3, 0)].ap())
    nc.scalar.dma_start(out=s_sb[:], in_=d_s.ap())
    nc.scalar.dma_start(out=vT_sb[:, 2048:3072], in_=d_vT.ap()[:, 2048:3072])
    nc.scalar.dma_start(out=vT_sb[:, 3072:4096], in_=d_vT.ap()[:, 3072:4096])
    nc.scalar.dma_start(out=dT_t[(1, 1)][:], in_=d_dataT[(1, 1)].ap())
    nc.scalar.dma_start(out=dT_t[(3, 1)][:], in_=d_dataT[(3, 1)].ap())
    if with_bias:
        bias2 = singles.tile([1, UNITS], BF16)
        nc.scalar.dma_start(out=bias2[:], in_=d_bias.ap())
        ones = singles.tile([1, P], BF16)
        nc.vector.memset(ones[:], 2.0)

    # ---- HAM warm-up ---------------------------------------------------
    # A contiguous >=3.4us burst fires the HAM un-throttle before the
    # first real matmul; keepers below then hold it warm through the
    # DMA-starved bt0 phase (they run inside what would be PE-idle gaps).
    wu_ps = o_psum.tile([P, MS], F32, tag="po", name="wu_ps")
    for _ in range(N_WARMUP_MM):
        nc.tensor.matmul(wu_ps[:, 0:256], lhsT=wu_a[:], rhs=wu_b[:], start=True, stop=True)

    def emit_keepers(n):
        for _ in range(n):
            nc.tensor.matmul(
                wu_ps[:, 0:256], lhsT=wu_a[:], rhs=wu_b[:], start=True, stop=True
            )

    # ---- compute stages ------------------------------------------------
    pd_t = {}
    smod_t = {}
    lowT_t = {}

    def emit_rank_mms(t, q):
        """mm1: pd[rc] += u_chunk.T @ dataT_chunk for k-chunks of slab q."""
        if q == 0:
            pd_t[t] = [
                du_psum.tile([P, BT], F32, tag="pd", name=f"pd{t}_{rc}")
                for rc in range(RC)
            ]
        for j in range(4):
            kc = q * 4 + j
            for rc in range(RC):
                nc.tensor.matmul(
                    pd_t[t][rc][:],
                    lhsT=u_t[q][:, j * RANK + rc * P : j * RANK + (rc + 1) * P],
                    rhs=dT_t[(q, t)][:, j * BT : (j + 1) * BT],
                    start=(kc == 0),
                    stop=(kc == KC - 1),
                )

    def emit_smod(t):
        """smod[rc] = s + ctx @ w ; independent of the data stream."""
        smod_t[t] = []
        for rc in range(RC):
            ps = o_psum.tile([P, BT], F32, tag="po", name=f"ps{t}_{rc}")
            for cc in range(CC):
                nc.tensor.matmul(
                    ps[:],
                    lhsT=w_sb[:, cc * RANK + rc * P : cc * RANK + (rc + 1) * P],
                    rhs=ctxT_t[t][:, cc * BT : (cc + 1) * BT],
                    start=(cc == 0),
                    stop=(cc == CC - 1),
                )
            smod = smodpool.tile([P, BT], F32, tag="smod", name=f"smod{t}_{rc}")
            nc.scalar.add(smod[:], ps[:], add=s_sb[:, rc : rc + 1])
            smod_t[t].append(smod)

    def emit_mul(t, bc):
        """lowT chunk bc = pd * smod on the vector engine (bf16 out)."""
        if bc == 0:
            lowT_t[t] = lowpool.tile([P, RC * BT], BF16, tag="lowT", name=f"lowT{t}")
        cols = slice(bc * P, (bc + 1) * P)
        for rc in range(RC):
            nc.vector.tensor_mul(
                out=lowT_t[t][:, rc * BT + bc * P : rc * BT + (bc + 1) * P],
                in0=pd_t[t][rc][:, cols],
                in1=smod_t[t][rc][:, cols],
            )

    def emit_out_stage(t, bc, store):
        """out rows = relu(low @ v.T [+ 2*bias]) for one 128-row chunk.

        store: engine for a whole-tile store, or a list of 4 engines for
        per-ms fine stores (used at the tail so the last store is small).
        """
        lowT = lowT_t[t]
        osb = opool.tile([P, UNITS], BF16, tag="osb", name=f"osb{t}_{bc}")
        rows = slice(t * BT + bc * P, t * BT + (bc + 1) * P)
        for ms in range(NMS):
            po = o_psum.tile([P, MS], F32, tag="po", name=f"po{t}_{bc}_{ms}")
            for rc in range(RC):
                nc.tensor.matmul(
                    po[:],
                    lhsT=lowT[:, rc * BT + bc * P : rc * BT + (bc + 1) * P],
                    rhs=vT_sb[:, ms * RC * MS + rc * MS : ms * RC * MS + (rc + 1) * MS],
                    start=(rc == 0),
                    stop=(rc == RC - 1) and not with_bias,
                )
            if with_bias:
                nc.tensor.matmul(
                    po[:],
                    lhsT=ones[:],
                    rhs=bias2[:, ms * MS : (ms + 1) * MS],
                    start=False,
                    stop=True,
                )
            sl = slice(ms * MS, (ms + 1) * MS)
            if ms % 2 == 0:
                nc.scalar.activation(
                    osb[:, sl], po[:], mybir.ActivationFunctionType.Relu
                )
            else:
                nc.vector.tensor_relu(out=osb[:, sl], in_=po[:])
            if isinstance(store, list):
                store[ms].dma_start(out=d_out.ap()[rows, sl], in_=osb[:, sl])
        if not isinstance(store, list):
            store.dma_start(out=d_out.ap()[rows, :], in_=osb[:])

    # Software pipeline: PE emission ordered by DMA arrival; bt1's rank
    # stage interleaves with bt0's output stage so the PE never waits on
    # the mul handoff, and stores alternate gpsimd/scalar rings (the
    # last two chunks fine-store over the by-then-idle sync ring).
    emit_rank_mms(0, 0)
    emit_keepers(8)
    emit_smod(0)
    emit_keepers(8)
    emit_rank_mms(0, 1)
    emit_keepers(8)
    emit_rank_mms(0, 2)
    emit_keepers(8)
    emit_rank_mms(0, 3)
    for bc in range(4):
        emit_mul(0, bc)
    emit_out_stage(0, 0, nc.scalar)
    emit_smod(1)
    emit_out_stage(0, 1, nc.scalar)
    emit_rank_mms(1, 0)
    emit_out_stage(0, 2, nc.scalar)
    emit_rank_mms(1, 1)
    emit_out_stage(0, 3, nc.scalar)
    emit_rank_mms(1, 2)
    emit_rank_mms(1, 3)
    for bc in range(4):
        emit_mul(1, bc)
    emit_out_stage(1, 0, nc.sync)
    emit_out_stage(1, 1, nc.scalar)
    emit_out_stage(1, 2, [nc.sync, nc.scalar, nc.sync, nc.scalar])
    emit_out_stage(1, 3, [nc.scalar, nc.sync, nc.scalar, nc.sync])


_CACHE = {}


def build(with_bias=False):
    key = ("nc", with_bias)
    if key in _CACHE:
        return _CACHE[key]
    nc = bacc.Bacc("TRN2", target_bir_lowering=False, debug=False)
    with tile.TileContext(nc) as tc, ExitStack() as ctx:
        _emit(nc, tc, ctx, with_bias)
    nc.compile()
    _CACHE[key] = nc
    return nc


def make_in_maps(data, context, u, s, v, w, bias, with_bias=False):
    bf = BF16_NP
    u = np.asarray(u, dtype=np.float32)
    s = np.asarray(s, dtype=np.float32)
    v = np.asarray(v, dtype=np.float32)
    w = np.asarray(w, dtype=np.float32)
    data = np.asarray(data, dtype=np.float32)
    context = np.asarray(context, dtype=np.float32)

    # u[(uq*4+j)*128+p, r] -> u_slab[uq][p, j*RANK+r]
    u_sl = u.reshape(4, 4, P, RANK).transpose(0, 2, 1, 3).reshape(4, P, 4 * RANK)
    u_sl = u_sl.astype(bf)
    # v[ms*512+mi, rc*128+p] -> vT_slab[p, ms*1024 + rc*512 + mi]
    vT_sl = (
        v.reshape(NMS, MS, RC, P).transpose(3, 0, 2, 1).reshape(P, RC * UNITS).astype(bf)
    )
    # w[cc*128+p, r] -> w_slab[p, cc*RANK+r]
    w_sl = w.reshape(CC, P, RANK).transpose(1, 0, 2).reshape(P, CC * RANK).astype(bf)
    # s[rc*128+p] -> s_slab[p, rc]
    s_sl = np.ascontiguousarray(s.reshape(RC, P).T)

    shared = {"s": s_sl, "vT": vT_sl, "w": w_sl}
    for uq in range(4):
        shared[f"u{uq}"] = u_sl[uq]
    if with_bias:
        shared["bias2"] = (2.0 * bias.astype(np.float32)).reshape(1, UNITS).astype(bf)

    in_maps = []
    for c in range(NCORES):
        sl = slice(c * NB, (c + 1) * NB)
        ds = data[sl]
        cs = context[sl]
        # data[t*BT+b, (q*4+j)*128+p] -> dataT_slab[q][t][p, j*BT+b]
        dt_sl = (
            ds.reshape(NBT, BT, NQ, 4, P)
            .transpose(2, 0, 4, 3, 1)
            .reshape(NQ, NBT, P, 4 * BT)
            .astype(bf)
        )
        # context[t*BT+b, cc*128+p] -> ctxT_slab[t][p, cc*BT+b]
        ct_sl = (
            cs.reshape(NBT, BT, CC, P)
            .transpose(0, 3, 2, 1)
            .reshape(NBT, P, CC * BT)
            .astype(bf)
        )
        m = dict(shared)
        for q in range(NQ):
            for t in range(NBT):
                if (q, t) == (0, 0):
                    m["dataT0_0a"] = np.ascontiguousarray(dt_sl[0, 0][:, : 2 * BT])
                    m["dataT0_0b"] = np.ascontiguousarray(dt_sl[0, 0][:, 2 * BT :])
                else:
                    m[f"dataT{q}_{t}"] = dt_sl[q, t]
        for t in range(NBT):
            m[f"ctxT{t}"] = ct_sl[t]
        in_maps.append(m)
    return in_maps


def kernel(data, context, u, s, v, w, bias):
    with_bias = bool(np.any(np.asarray(bias)))
    nc = build(with_bias)
    in_maps = make_in_maps(data, context, u, s, v, w, bias, with_bias=with_bias)
    res = run_bass_kernel_spmd(nc, in_maps, core_ids=list(range(NCORES)))
    return np.concatenate(
        [np.asarray(r["out"]).astype(np.float32) for r in res.results], axis=0
    )
